# revision 12
# baseline (speedup 1.0000x reference)
"""Trainium2 Bass kernel for dual-input complement-softmax attention.

Reference computation (B=4, C=256, H=W=64, N=4096, INTER=128):
  q1 = relu(bn(conv1x1(x1, Wq)))   -> (B, N, 128)
  k2 = relu(bn(conv1x1(x2, Wk)))   -> (B, 128, N)
  v1 = relu(bn(conv1x1(x1, Wv)))   -> (B, N, 128)
  v2 = relu(bn(conv1x1(x2, Wv)))   -> (B, N, 128)
  A  = 1 - softmax(q1 k2 / sqrt(128))
  y  = concat(A v2, A v1)          -> (B, N, 256)
  out = relu(bn(conv1x1(y, Wp)))   -> (B, 256, H, W)

Key algebra used on-device (per batch b, query rows n, key rows m):
  A = 1 - P/r  with P = exp(dots), r = rowsum(P)
  out[n, o] = relu( h[o] - rn[n] * F[n, o] )
    F[n, o]  = sum_m P[n, m] * VW[m, o]     (VW = [v2|v1] @ Wp_eff^T)
    h[o]     = Wp_eff @ [sumv2|sumv1] + bp_eff
    rn[n]    = 1 / r[n]   (r obtained free as a ones-column of the F matmul)

The m (key) axis is permuted per core (query half first) identically for x1
and x2; every consumer reduces over m, so the permutation cancels and lets
q share the x1 tiles (no separate q input).

Sharding: 8 cores = 4 batches x 2 query-row halves. Each core computes its
own (256, 2048) slab of the output; no collectives.
"""

import numpy as np

B, C, HH, WW = 4, 256, 64, 64
N = HH * WW        # 4096 keys per batch
NQ = N // 2        # 2048 query rows per core
INTER = 128
OUT = 256
NCORES = 8

_NC_CACHE = {}


def _build_nc():
    import concourse.bacc as bacc
    import concourse.mybir as mybir
    import concourse.tile as tile

    f32 = mybir.dt.float32
    f32r = mybir.dt.float32r
    bf16 = mybir.dt.bfloat16
    A = mybir.AluOpType
    AF = mybir.ActivationFunctionType
    X = mybir.AxisListType.X

    nc = bacc.Bacc(None, target_bir_lowering=False)

    x1 = nc.dram_tensor("x1", [C, N], f32r, kind="ExternalInput")
    x2 = nc.dram_tensor("x2", [C, N], f32r, kind="ExternalInput")
    wqkv = nc.dram_tensor("wqkv", [3 * C, INTER], f32r, kind="ExternalInput")
    wpT = nc.dram_tensor("wpT", [OUT, OUT], f32r, kind="ExternalInput")
    bqkv = nc.dram_tensor("bqkv", [3 * INTER, 1], f32, kind="ExternalInput")
    bp_row = nc.dram_tensor("bp_row", [1, OUT], f32, kind="ExternalInput")
    ident = nc.dram_tensor("ident", [128, 128], f32, kind="ExternalInput")

    out = nc.dram_tensor("out", [OUT, NQ], f32, kind="ExternalOutput")
    out_r = out[:].rearrange("(a p) n -> p a n", p=128)

    MCHUNKS = N // 128           # 32 key chunks
    SB = 512                     # query superblock
    NSB = NQ // SB               # 4

    with tile.TileContext(nc) as tc:
        with (
            tc.tile_pool(name="persist", bufs=1) as persist,
            tc.tile_pool(name="xp", bufs=6) as xp,
            tc.tile_pool(name="vp", bufs=3) as vp,
            tc.tile_pool(name="ep", bufs=36) as ep,
            tc.tile_pool(name="zp", bufs=2) as zp,
            tc.tile_pool(name="ps", bufs=2, space="PSUM") as ps,
            tc.tile_pool(name="fp", bufs=2, space="PSUM") as fp,
            tc.tile_pool(name="sp", bufs=2, space="PSUM") as sp,
        ):
            # ---- persistent tiles ----
            wqkv_sb = persist.tile([128, 6, INTER], f32r)
            wp_sb = persist.tile([128, 2, OUT], f32r)
            bqkv_sb = persist.tile([128, 3], f32)
            bp_sb = persist.tile([1, OUT], f32)
            id_sb = persist.tile([128, 128], f32)
            k2_sb = persist.tile([128, N], f32r)            # [c, m]
            q_sb = persist.tile([128, NQ], f32r)            # [c, n]
            v2_sb = persist.tile([128, N], f32r)            # [c, m]
            vw_sb = persist.tile([128, MCHUNKS, 258], bf16)  # [m%128, m//128, o|1,1]
            ones_f = persist.tile([128, MCHUNKS, 2], f32)
            sv1_part = persist.tile([128, 4], f32)
            h_row = persist.tile([1, OUT], f32)
            h_bc = persist.tile([128, OUT], f32)

            def xpair(src, s2):
                """One 1024-col slab of src as 2 [128,1024] tiles (ch chunks)."""
                sl = slice(s2 * 1024, (s2 + 1) * 1024)
                c0 = xp.tile([128, 1024], f32r, tag="x", name="x_c0")
                c1 = xp.tile([128, 1024], f32r, tag="x", name="x_c1")
                nc.sync.dma_start(c0[:], src[0:128, sl])
                nc.sync.dma_start(c1[:], src[128:256, sl])
                return c0, c1

            def proj_pair(a, c0, c1):
                """psum[:, 0:1024] = W[a].T @ x(slab); W chunk pair a."""
                psum = ps.tile([128, 1024], f32, tag="ps", name="pp")
                for hf in (0, 1):
                    o = hf * 512
                    nc.tensor.matmul(psum[:, o:o + 512], wqkv_sb[:, 2 * a],
                                     c0[:, o:o + 512], start=True, stop=False)
                    nc.tensor.matmul(psum[:, o:o + 512], wqkv_sb[:, 2 * a + 1],
                                     c1[:, o:o + 512], start=False, stop=True)
                return psum

            # ---- DMA front: x1 slab0 (for q), weights, then x2 slabs ----
            x1_0 = xpair(x1, 0)
            nc.sync.dma_start(wqkv_sb[:], wqkv[:].rearrange("(a p) i -> p a i", p=128))
            nc.sync.dma_start(wp_sb[:], wpT[:].rearrange("(a p) o -> p a o", p=128))
            nc.sync.dma_start(bqkv_sb[:], bqkv[:].rearrange("(a p) o -> p (a o)", p=128))
            nc.sync.dma_start(bp_sb[:], bp_row[:])
            nc.sync.dma_start(id_sb[:], ident[:])
            nc.vector.memset(ones_f[:], 1.0)
            nc.vector.tensor_copy(vw_sb[:, :, 256:258], ones_f[:])

            def vw_mms(v1_t, s2):
                """8 VW chunks for the 1024-wide m-slab s2 (v2 from v2_sb)."""
                for mc in range(8):
                    j = s2 * 8 + mc
                    vwps = sp.tile([128, 258], f32, tag="small", name="vwps")
                    nc.tensor.matmul(vwps[:, 0:256], v2_sb[:, j * 128:(j + 1) * 128],
                                     wp_sb[:, 0], start=True, stop=False)
                    nc.tensor.matmul(vwps[:, 0:256], v1_t[:, mc * 128:(mc + 1) * 128],
                                     wp_sb[:, 1], start=False, stop=True)
                    nc.vector.tensor_copy(vw_sb[:, j, 0:256], vwps[:, 0:256])

            def v1_block(s2, xt, with_q):
                c0, c1 = xt
                sl = slice(s2 * 1024, (s2 + 1) * 1024)
                if with_q:
                    qps = proj_pair(0, c0, c1)
                    nc.vector.tensor_scalar(q_sb[:, sl], qps[:], bqkv_sb[:, 0:1], 0.0,
                                            A.add, A.max)
                v1ps = proj_pair(2, c0, c1)
                v1_t = vp.tile([128, 1024], f32r, tag="v1")
                nc.vector.tensor_scalar(v1_t[:], v1ps[:], bqkv_sb[:, 2:3], 0.0,
                                        A.add, A.max)
                nc.vector.tensor_reduce(sv1_part[:, s2:s2 + 1], v1_t[:], X, A.add)
                vw_mms(v1_t, s2)

            exp_map = {sb: [None] * (MCHUNKS // 2) for sb in range(NSB)}

            def dots_slab(sbs, s2):
                """dps+exp for m-chunks of slab s2, for each query superblock."""
                for sb in sbs:
                    nsl = slice(sb * SB, (sb + 1) * SB)
                    for jl in range(4):
                        jj = s2 * 4 + jl
                        dps = ps.tile([128, 1024], f32, tag="ps", name="dps")
                        for u in (0, 1):
                            j = jj * 2 + u
                            nc.tensor.matmul(dps[:, u * 512:(u + 1) * 512],
                                             k2_sb[:, j * 128:(j + 1) * 128],
                                             q_sb[:, nsl], start=True, stop=True)
                        et = ep.tile([128, 1024], bf16, tag="exp", name="et")
                        nc.scalar.activation(et[:], dps[:], AF.Exp)
                        exp_map[sb][jj] = et

            def pv_out(sb):
                tiles = exp_map.pop(sb)
                for t in range(SB // 128):
                    nt = sb * 4 + t
                    fps = fp.tile([128, 258], f32, tag="F", name="fps")
                    for j in range(MCHUNKS):
                        jj, u = j // 2, j % 2
                        c0 = u * 512 + t * 128
                        nc.tensor.matmul(fps[:], tiles[jj][:, c0:c0 + 128],
                                         vw_sb[:, j, 0:258],
                                         start=(j == 0), stop=(j == MCHUNKS - 1))
                    rn = zp.tile([128, 1], f32, tag="rn")
                    nc.vector.reciprocal(rn[:], fps[:, 256:257])
                    t2 = zp.tile([128, OUT], f32, tag="t2")
                    nc.vector.scalar_tensor_tensor(t2[:], fps[:, 0:256], rn[:],
                                                   h_bc[:], A.mult, A.subtract)
                    z = zp.tile([128, OUT], f32, tag="z")
                    nc.vector.tensor_scalar(z[:], t2[:], -1.0, 0.0, A.mult, A.max)
                    ztps = sp.tile([128, 258], f32, tag="small", name="ztps")
                    nc.tensor.transpose(ztps[:, 0:128], z[:, 0:128], id_sb[:])
                    nc.tensor.transpose(ztps[:, 128:256], z[:, 128:256], id_sb[:])
                    zt = zp.tile([128, 2, 128], f32, tag="zt")
                    nc.vector.tensor_copy(zt[:], ztps[:, 0:256])
                    nc.sync.dma_start(out_r[:, :, nt * 128:(nt + 1) * 128], zt[:])

            # ---- phase 0/1 interleave: k/v2 per slab + early dots for sb0/sb1 ----
            qps0 = proj_pair(0, x1_0[0], x1_0[1])
            nc.vector.tensor_scalar(q_sb[:, 0:1024], qps0[:], bqkv_sb[:, 0:1], 0.0,
                                    A.add, A.max)
            for s2 in range(4):
                c0, c1 = xpair(x2, s2)
                sl = slice(s2 * 1024, (s2 + 1) * 1024)
                kps = proj_pair(1, c0, c1)
                nc.vector.tensor_scalar(k2_sb[:, sl], kps[:], bqkv_sb[:, 1:2], 0.0,
                                        A.add, A.max)
                vps = proj_pair(2, c0, c1)
                nc.vector.tensor_scalar(v2_sb[:, sl], vps[:], bqkv_sb[:, 2:3], 0.0,
                                        A.add, A.max)
                if s2 == 0:
                    v1_block(0, x1_0, with_q=False)
                dots_slab((0,), s2)

            for s2 in range(1, 4):
                xt = xpair(x1, s2)
                v1_block(s2, xt, with_q=(s2 == 1))

            # ---- sumv totals, h ----
            sv2f = zp.tile([128, 1], f32, tag="svf")
            sv1f = zp.tile([128, 1], f32, tag="svf")
            nc.vector.tensor_reduce(sv2f[:], v2_sb[:], X, A.add)
            nc.vector.tensor_reduce(sv1f[:], sv1_part[:], X, A.add)
            hps = sp.tile([128, 258], f32, tag="small", name="hps")
            nc.tensor.matmul(hps[0:1, 0:256], sv2f[:], wp_sb[:, 0].bitcast(f32),
                             start=True, stop=False)
            nc.tensor.matmul(hps[0:1, 0:256], sv1f[:], wp_sb[:, 1].bitcast(f32),
                             start=False, stop=True)
            nc.vector.tensor_tensor(h_row[:], hps[0:1, 0:256], bp_sb[:], A.add)
            nc.gpsimd.partition_broadcast(h_bc[:], h_row[:])

            # ---- phase 1 steady state: PV(k) overlaps dots(k+1) exps ----
            for s2 in range(4):
                dots_slab((1,), s2)
            pv_out(0)
            for s2 in range(4):
                dots_slab((2,), s2)
            pv_out(1)
            for s2 in range(4):
                dots_slab((3,), s2)
            pv_out(2)
            pv_out(3)

    nc.compile()
    return nc


def _host_prep(inputs):
    s_attn = np.float32(INTER ** -0.5)
    x1 = np.ascontiguousarray(inputs["x1"], dtype=np.float32).reshape(B, C, N)
    x2 = np.ascontiguousarray(inputs["x2"], dtype=np.float32).reshape(B, C, N)

    def eff(Wn, bn, sn, tn, extra=np.float32(1.0)):
        Wm = np.asarray(inputs[Wn], np.float32)
        bb = np.asarray(inputs[bn], np.float32)
        ss = np.asarray(inputs[sn], np.float32)
        tt = np.asarray(inputs[tn], np.float32)
        W_eff = (ss[:, None] * Wm) * extra
        b_eff = (ss * bb + tt) * extra
        return np.ascontiguousarray(W_eff.T), b_eff

    wqT, bqe = eff("Wq", "bq", "sq", "tq", s_attn)
    wkT, bke = eff("Wk", "bk", "sk", "tk")
    wvT, bve = eff("Wv", "bv", "sv", "tv")
    wpT, bpe = eff("Wp", "bp", "sp", "tp")

    common = dict(
        wqkv=np.ascontiguousarray(np.concatenate([wqT, wkT, wvT], axis=0)),
        wpT=wpT,
        bqkv=np.concatenate([bqe, bke, bve]).reshape(3 * INTER, 1),
        bp_row=bpe.reshape(1, OUT),
        ident=np.eye(128, dtype=np.float32),
    )
    in_maps = []
    for c in range(NCORES):
        b, half = c // 2, c % 2
        # m-axis permutation: own query half first (identical for x1 and x2,
        # so all sum-over-m quantities are unchanged)
        perm = (np.r_[NQ:N, 0:NQ] if half else np.r_[0:N]).astype(np.intp)
        in_maps.append(dict(
            x1=np.ascontiguousarray(x1[b][:, perm]),
            x2=np.ascontiguousarray(x2[b][:, perm]),
            **common,
        ))
    return in_maps


def kernel(**inputs):
    from concourse.bass_utils import run_bass_kernel_spmd

    if "nc" not in _NC_CACHE:
        _NC_CACHE["nc"] = _build_nc()
    nc = _NC_CACHE["nc"]

    in_maps = _host_prep(inputs)
    res = run_bass_kernel_spmd(nc, in_maps, core_ids=list(range(NCORES)))

    full = np.empty((B, OUT, N), dtype=np.float32)
    for c in range(NCORES):
        b, half = c // 2, c % 2
        full[b][:, half * NQ:(half + 1) * NQ] = res.results[c]["out"]
    return full.reshape(B, OUT, HH, WW)


if __name__ == "__main__":
    rng = np.random.default_rng(0)
    fake = {}
    fake["x1"] = rng.standard_normal((B, C, HH, WW), dtype=np.float32)
    fake["x2"] = rng.standard_normal((B, C, HH, WW), dtype=np.float32)
    for k, oc in (("q", INTER), ("k", INTER), ("v", INTER), ("p", OUT)):
        ic = C if k != "p" else 2 * INTER
        fake["W" + k] = rng.standard_normal((oc, ic), dtype=np.float32) * ic ** -0.5
        fake["b" + k] = np.zeros(oc, np.float32)
        fake["s" + k] = rng.uniform(0.5, 1.5, oc).astype(np.float32)
        fake["t" + k] = rng.standard_normal(oc, dtype=np.float32) * 0.1
    o = kernel(**fake)
    print("kernel ran, out shape", o.shape)
